# revision 59
# baseline (speedup 1.0000x reference)
"""Trainium2 Bass kernel for nn_MBSFeedForward (moe_routing).

Reference semantics (per token t with class c = type_seq[t]):
  c == 0:  out = LN_out(x_t)
  c >= 1:  e = c-1 (expert)
           u = GELU(x_t @ W1_e + b1_e) @ W2_e + b2_e
           v = LN_e(u + x_t)          (per-expert ln_g/ln_b)
           out = LN_out(v + x_t)      (out_g/out_b)

Sharding (host does the routing):
  - 4 experts x 2 cores each; each core runs half of its expert's tokens
    through the full FFN chain. class-0 tokens are split evenly over all
    8 cores and only go through the outer LayerNorm. No collectives.

Device kernel (per core, SPMD):
  - GEMM1 produces GELU inputs in [F, tok] layout (F on partitions); GEMM2
    consumes them as stationary and produces U in [tok, H] layout; no
    transposes (host supplies x pre-transposed/packed).
  - GEMMs run in fp8e4m3 with DoubleRow perf mode (2 contraction rows per
    partition) by default; bf16 fallback per GEMM.
  - The first residual (+x) is folded into the GEMM2 PSUM accumulation via
    an f32r identity matmul, so the LN1 chain reads PSUM directly.
  - LayerNorm rsqrt runs on DVE with a bit-trick seed + 1 Newton step —
    no Sqrt on the Activation engine, so the Gelu activation table is
    never swapped out (act-table reloads cost 1.3us each).
  - GELU is applied once per kf-pair ([128, 2, TCH] tiles) to amortize the
    Activation engine's fixed per-instruction overhead.
  - The kp-loop is software-pipelined: GEMM1(+GELU) of chunk c+PIPE is
    interleaved between GEMM2 calls of chunk c, so the PE stream stays
    busy through the weight-stream startup and across chunk boundaries;
    class-0 (LN-only) tiles ride along in the earliest chunks.
  - In the tail region (no GEMM1 left) the wide LN1 normalize moves to
    the Activation engine (Identity with bias=-mean*rs, scale=rs) and the
    residual add to DVE, shortening the final serial LN chains.
  - Residuals/outputs are bf16 (DVE 2x modes, half DMA); weights stream
    on the SP queue so they never block GELU issue on the Act queue.
  - Affine LN params / b1 / b2 that are identity (ones/zeros) are elided
    at build time (detected on host from the actual inputs).
"""

import numpy as np
import ml_dtypes

import concourse.bass as bass
import concourse.tile as tile
from concourse import bacc
from concourse import mybir
from concourse.bass_utils import run_bass_kernel_spmd

F32 = mybir.dt.float32
F32R = mybir.dt.float32r
BF16 = mybir.dt.bfloat16
FP8 = mybir.dt.float8e4
I32 = mybir.dt.int32
ALU = mybir.AluOpType
DR = mybir.MatmulPerfMode.DoubleRow

P = 128
H = 768
F = 3072
KH = H // P    # 6
KF = F // P    # 24
NKP = KF // 2  # 12 kf-pairs
TCH = 256      # tokens per chunk
TPT = TCH // P
NCORES = 8
EPS = 1e-12
# rsqrt magic, pre-shifted for w = z/2 input: 0x5f375a86 - 0x00400000
MAGIC = 0x5EF75A86

G1_DT = "fp8"   # "bf16" | "fp8"
G2_DT = "fp8"
# power-of-2 pre-scales applied before fp8 quantization (exact to undo):
# keeps N(0, 1/768)-ish weights out of e4m3's subnormal range. The GELU
# activation divides by SCL_X*SCL_W1; GEMM2's output scale SCL_W2 is folded
# into the identity-residual matmul and normalized away by LN1 (LN is
# scale-invariant; the affine params apply post-normalization).
SCL_X = 16.0
SCL_W1 = 256.0
SCL_W2 = 512.0

# swappable for CoreSim validation (Gelu not implemented in the interpreter)
ACT_FUNC = mybir.ActivationFunctionType.Gelu


def round_f32r(a: np.ndarray) -> np.ndarray:
    """Round fp32 to fp32r (e8m11: low 12 mantissa bits dropped, RNE)."""
    u = np.ascontiguousarray(a, dtype=np.float32).view(np.uint32)
    r = (u + np.uint32(0x7FF) + ((u >> np.uint32(12)) & np.uint32(1))) & np.uint32(0xFFFFF000)
    return r.view(np.float32)


def build_nc(cap: int, cap0: int, repeat: int = 1, *,
             g1: str = G1_DT, g2: str = G2_DT,
             triv_b1: bool = True, triv_b2: bool = True,
             triv_aff1: bool = True, triv_aff2: bool = True,
             rsqrt_pool: bool = False, skip_tiles: int = 0,
             ps1_bufs: int = 2, psu_bufs: int = 3, ha_bufs: int = 50,
             work_bufs: int = 4, xin_bufs: int = 4, xres_bufs: int = 4,
             stat_bufs: int = 6) -> bass.Bass:
    """Build the per-core Bass module for `cap` routed tokens and `cap0`
    LN-only tokens. repeat > 1 re-runs the computation for slope timing."""
    assert cap % TCH == 0 and cap0 % P == 0
    CC = cap // TCH
    nc = bacc.Bacc()

    d_xr = nc.dram_tensor("xr", [cap, H], BF16, kind="ExternalInput")
    d_x0 = nc.dram_tensor("x0", [cap0, H], BF16, kind="ExternalInput")
    d_ident = nc.dram_tensor("ident", [P, P], BF16, kind="ExternalInput")
    if g1 == "fp8":
        d_xrT = nc.dram_tensor("xrT", [P, CC, 3, 2, TCH], FP8, kind="ExternalInput")
        d_w1 = nc.dram_tensor("w1", [H, F], FP8, kind="ExternalInput")
    else:
        d_xrT = nc.dram_tensor("xrT", [H, cap], BF16, kind="ExternalInput")
        d_w1 = nc.dram_tensor("w1", [H, F], BF16, kind="ExternalInput")
    g2dt = FP8 if g2 == "fp8" else BF16
    d_w2 = nc.dram_tensor("w2", [F, H], g2dt, kind="ExternalInput")
    if not triv_b1:
        d_b1 = nc.dram_tensor("b1", [F], F32, kind="ExternalInput")
    if not triv_b2:
        d_b2 = nc.dram_tensor("b2", [H], g2dt, kind="ExternalInput")
    if not triv_aff1:
        d_lng = nc.dram_tensor("lng", [H], F32, kind="ExternalInput")
        d_lnb = nc.dram_tensor("lnb", [H], F32, kind="ExternalInput")
    if not triv_aff2:
        d_outg = nc.dram_tensor("outg", [H], F32, kind="ExternalInput")
        d_outb = nc.dram_tensor("outb", [H], F32, kind="ExternalInput")
    d_yr = nc.dram_tensor("yr", [cap, H], BF16, kind="ExternalOutput")
    d_y0 = nc.dram_tensor("y0", [cap0, H], BF16, kind="ExternalOutput")

    from contextlib import ExitStack
    with tile.TileContext(nc) as tc, ExitStack() as ctx:
        singles = ctx.enter_context(tc.tile_pool(name="singles", bufs=1))
        xin = ctx.enter_context(tc.tile_pool(name="xin", bufs=xin_bufs))
        xres = ctx.enter_context(tc.tile_pool(name="xres", bufs=xres_bufs))
        hpool = ctx.enter_context(tc.tile_pool(name="hact", bufs=ha_bufs))
        work = ctx.enter_context(tc.tile_pool(name="work", bufs=work_bufs))
        stat = ctx.enter_context(tc.tile_pool(name="stat", bufs=stat_bufs))
        ps1p = ctx.enter_context(tc.tile_pool(name="ps1", bufs=ps1_bufs, space="PSUM"))
        psUp = ctx.enter_context(tc.tile_pool(name="psU", bufs=psu_bufs, space="PSUM"))

        # --- resident weights / constants, streamed in consumption order ---
        if g1 == "fp8":
            w1sb = singles.tile([P, 3, 2, F], FP8)
            w1_v = d_w1[:].rearrange("(hp two ki) f -> ki hp two f", ki=P, two=2)
        else:
            w1sb = singles.tile([P, KH, F], BF16)
            w1_v = d_w1[:].rearrange("(ko ki) f -> ki ko f", ki=P)
        if g2 == "fp8":
            w2sb = singles.tile([P, NKP, 2, H], FP8)
            w2_v = d_w2[:].rearrange("(kp two ki) h -> ki kp two h", ki=P, two=2)
        else:
            w2sb = singles.tile([P, KF, H], BF16)
            w2_v = d_w2[:].rearrange("(ko ki) h -> ki ko h", ki=P)
        for fb in range(F // 512):
            if g1 == "fp8":
                nc.sync.dma_start(
                    w1sb[:, :, :, fb * 512:(fb + 1) * 512],
                    w1_v[:, :, :, fb * 512:(fb + 1) * 512],
                )
            else:
                nc.sync.dma_start(
                    w1sb[:, :, fb * 512:(fb + 1) * 512],
                    w1_v[:, :, fb * 512:(fb + 1) * 512],
                )
        if g2 == "fp8":
            for kp in range(NKP):
                nc.sync.dma_start(w2sb[:, kp], w2_v[:, kp])
        else:
            for ko in range(KF):
                nc.sync.dma_start(w2sb[:, ko], w2_v[:, ko])

        identsb = singles.tile([P, P], BF16)
        nc.sync.dma_start(identsb, d_ident[:])

        if not triv_b1:
            b1sb = singles.tile([P, KF], F32)
            nc.sync.dma_start(b1sb, d_b1[:].rearrange("(o p) -> p o", p=P))
        if not triv_b2:
            ones_t = singles.tile([1, P], g2dt)
            nc.vector.memset(ones_t, 1.0)
            b2row = singles.tile([1, H], g2dt)
            nc.sync.dma_start(b2row, d_b2[:][None, :])

        def bc_tile(d, nm):
            t = singles.tile([P, H], F32, tag=nm, name=nm)
            nc.gpsimd.dma_start(t, d[:][None, :].to_broadcast([P, H]))
            return t

        lngbc = None if triv_aff1 else bc_tile(d_lng, "lngbc")
        lnbbc = None if triv_aff1 else bc_tile(d_lnb, "lnbbc")
        outgbc = None if triv_aff2 else bc_tile(d_outg, "outgbc")
        outbbc = None if triv_aff2 else bc_tile(d_outb, "outbbc")

        nr_eng = nc.gpsimd if rsqrt_pool else nc.vector

        def emit_rsqrt(w, tag, n=1):
            """y ~= 1/sqrt(2w) elementwise for w = (var+eps)/2, [P,n] f32
            SBUF: bit-trick seed + one (tuned) Newton step."""
            y = stat.tile([P, n], F32, tag=tag + "y", name=tag + "y")
            si = stat.tile([P, n], I32, tag=tag + "i", name=tag + "i")
            nr_eng.tensor_scalar(si, w.bitcast(I32), 1, -1,
                                 op0=ALU.logical_shift_right,
                                 op1=ALU.bitwise_xor)
            nr_eng.tensor_scalar(y.bitcast(I32), si, MAGIC + 1, None,
                                 op0=ALU.add)
            t = stat.tile([P, n], F32, tag=tag + "t", name=tag + "t")
            nr_eng.tensor_tensor(t, y, y, op=ALU.mult)
            if n == 1:
                nr_eng.tensor_scalar(t, t, w, 1.5, op0=ALU.mult,
                                     op1=ALU.subtract)
                nr_eng.scalar_tensor_tensor(y, t, -1.0, y, op0=ALU.mult,
                                            op1=ALU.mult)
            else:
                nr_eng.tensor_tensor(t, t, w, op=ALU.mult)
                nr_eng.tensor_scalar(t, t, -1.0, 1.5, op0=ALU.mult,
                                     op1=ALU.add)
                nr_eng.tensor_tensor(y, t, y, op=ALU.mult)
            return y

        def layernorm(src, dst, gbc, bbc, tag, act_norm=False,
                      act_stats=False):
            """dst = LN(src) (affine applied if gbc given). src may be PSUM
            or SBUF [P, H]; dst must be SBUF (may alias src if SBUF).
            act_norm routes the wide normalize op through the Activation
            engine (Copy with bias=-mean*rs, scale=rs); act_stats computes
            the sums on the Activation engine too (Identity/Square with
            accum_out) — both used near the kernel tail where the DVE chain
            is the critical path."""
            mv = stat.tile([P, 2], F32, tag=tag + "mv", name=tag + "mv")
            if act_stats:
                sy = stat.tile([P, 1], F32, tag=tag + "sy", name=tag + "sy")
                sq = stat.tile([P, 1], F32, tag=tag + "sq", name=tag + "sq")
                scr = work.tile([P, H], BF16, tag="scr", name=tag + "scr")
                nc.scalar.activation(scr, src,
                                     mybir.ActivationFunctionType.Identity,
                                     accum_out=sy)
                nc.scalar.activation(scr, src,
                                     mybir.ActivationFunctionType.Square,
                                     accum_out=sq)
                nc.vector.tensor_scalar(mv[:, 0:1], sy, 1.0 / H, None,
                                        op0=ALU.mult)
                t0 = stat.tile([P, 1], F32, tag=tag + "t0", name=tag + "t0")
                nc.vector.tensor_tensor(t0, mv[:, 0:1], mv[:, 0:1],
                                        op=ALU.mult)
                nc.vector.scalar_tensor_tensor(mv[:, 1:2], sq, 1.0 / H, t0,
                                               op0=ALU.mult, op1=ALU.subtract)
            else:
                st = stat.tile([P, 2, 6], F32, tag=tag + "st", name=tag + "st")
                nc.vector.bn_stats(st[:, 0], src[:, 0:512])
                nc.vector.bn_stats(st[:, 1], src[:, 512:H])
                nc.vector.bn_aggr(mv, st)
            w = stat.tile([P, 1], F32, tag=tag + "w", name=tag + "w")
            nc.vector.tensor_scalar(w, mv[:, 1:2], 0.5, EPS * 0.5,
                                    op0=ALU.mult, op1=ALU.add)
            rs = emit_rsqrt(w, tag)
            if act_norm:
                mrs = stat.tile([P, 1], F32, tag=tag + "mr", name=tag + "mr")
                nc.vector.scalar_tensor_tensor(mrs, mv[:, 0:1], -1.0, rs,
                                               op0=ALU.mult, op1=ALU.mult)
                nc.scalar.activation(dst, src,
                                     mybir.ActivationFunctionType.Identity,
                                     bias=mrs, scale=rs)
            else:
                nc.vector.tensor_scalar(dst, src, mv[:, 0:1], rs,
                                        op0=ALU.subtract, op1=ALU.mult)
            if gbc is not None:
                nc.gpsimd.tensor_mul(dst, dst, gbc)
                nc.gpsimd.tensor_add(dst, dst, bbc)

        def class0_tile(t):
            """One [P, H] class-0 tile: load, outer LN, store (bf16 out)."""
            x0t = work.tile([P, H], BF16, tag="x0t", name="x0t")
            nc.sync.dma_start(x0t, d_x0[t * P:(t + 1) * P, :])
            o0 = work.tile([P, H], BF16, tag="ro", name="x0o")
            layernorm(x0t, o0, outgbc, outbbc, "l0")
            nc.sync.dma_start(d_y0[t * P:(t + 1) * P, :], o0)

        if g1 == "bf16":
            xrT_v = d_xrT[:].rearrange("(ko ki) t -> ki ko t", ki=P)

        # class-0 tiles are spread across the EARLIEST routed chunks (their
        # LN work rides along while the FFN pipeline is weight-load-bound and
        # never piles up at the tail).
        N0 = cap0 // P

        def load_xin(c, eng=None):
            eng = eng or nc.gpsimd
            if g1 == "fp8":
                t = xin.tile([P, 3, 2, TCH], FP8, tag="xin", name="xin")
                eng.dma_start(t, d_xrT[:, c])
            else:
                t = xin.tile([P, KH, TCH], BF16, tag="xin", name="xin")
                eng.dma_start(t, xrT_v[:, :, c * TCH:(c + 1) * TCH])
            return t

        gscale = 1.0 / (SCL_X * SCL_W1) if g1 == "fp8" else 1.0

        def g1_gelu(kq, xrTt, wd=TCH):
            """GEMM1 + GELU for one kf-pair over `wd` tokens (wd < TCH when
            the chunk's trailing tiles are pure padding); returns ha."""
            ps1 = ps1p.tile([P, 2, TCH], F32, tag="ps1")
            for q in range(2):
                kf = 2 * kq + q
                if g1 == "fp8":
                    for hp in range(3):
                        nc.tensor.matmul(
                            ps1[:, q, :wd],
                            w1sb[:, hp, :, kf * P:(kf + 1) * P],
                            xrTt[:, hp, :, :wd],
                            start=(hp == 0), stop=(hp == 2),
                            perf_mode=DR,
                        )
                else:
                    for kh in range(KH):
                        nc.tensor.matmul(
                            ps1[:, q, :wd],
                            w1sb[:, kh, kf * P:(kf + 1) * P],
                            xrTt[:, kh, :wd],
                            start=(kh == 0), stop=(kh == KH - 1),
                        )
            # one gelu per pair (b1 is per-F-tile: only fusable when 0)
            ha = hpool.tile([P, 2, TCH], g2dt, tag="ha")
            if triv_b1:
                nc.scalar.activation(ha[:, :, :wd], ps1[:, :, :wd], ACT_FUNC,
                                     scale=gscale)
            else:
                for q in range(2):
                    nc.scalar.activation(
                        ha[:, q, :wd], ps1[:, q, :wd], ACT_FUNC,
                        bias=b1sb[:, 2 * kq + q:2 * kq + q + 1], scale=gscale,
                    )
            return ha

        def g2(kp, haq, psUs, ms=None):
            s2 = 0
            for m in (range(TPT) if ms is None else ms):
                for h0, h1 in ((0, 512), (512, H)):
                    if g2dt == FP8:
                        nc.tensor.matmul(
                            psUs[m][:, h0:h1],
                            haq[:, s2:s2 + 2, m * P:(m + 1) * P],
                            w2sb[:, kp, :, h0:h1],
                            start=(kp == 0), stop=False,
                            perf_mode=DR,
                        )
                    else:
                        for q in (0, 1):
                            nc.tensor.matmul(
                                psUs[m][:, h0:h1],
                                haq[:, s2 + q, m * P:(m + 1) * P],
                                w2sb[:, 2 * kp + q, h0:h1],
                                start=(kp == 0 and q == 0), stop=False,
                            )

        # --- routed tokens: software-pipelined FFN chain. GEMM1(+GELU) of
        # chunk c+1 is interleaved between GEMM2 calls of chunk c so the PE
        # stream never serializes on a full chunk boundary.
        def close_m(c, m, psUs, xrts):
            """b2 + identity-residual matmuls closing psU[m]'s group."""
            for h0, h1 in ((0, 512), (512, H)):
                if not triv_b2:
                    nc.tensor.matmul(
                        psUs[m][:, h0:h1], ones_t, b2row[:, h0:h1],
                        start=False, stop=False,
                    )
                nc.tensor.matmul(
                    psUs[m][:, h0:h1], identsb, xrts[m][:, h0:h1],
                    start=False, stop=(h0 == 512),
                )

        def norm_m(c, m, psUs, xrts, act_norm, act_stats=False):
            tok0 = c * TCH + m * P
            r = work.tile([P, H], BF16, tag="r", name=f"r_{m}")
            layernorm(psUs[m], r, lngbc, lnbbc, "l1", act_norm=act_norm,
                      act_stats=act_stats)
            # in the tail region DVE has slack and bf16 gets its 2x mode;
            # mid-stream the add runs on the otherwise-idle Pool engine
            (nc.vector if act_norm else nc.gpsimd).tensor_add(r, r, xrts[m])
            ro = work.tile([P, H], BF16, tag="ro", name=f"ro_{m}")
            layernorm(r, ro, outgbc, outbbc, "l2")
            nc.sync.dma_start(d_yr[tok0:tok0 + P, :], ro)

        def ln_level_batched(srcs, dsts, gbc, bbc, tag):
            """One LN level over TPT tiles at once: per-tile stats, one
            shared [P, TPT] rsqrt chain, per-tile Activation normalizes."""
            mvb = stat.tile([P, TPT, 2], F32, tag=tag + "mv", name=tag + "mv")
            for m in range(TPT):
                st = stat.tile([P, 2, 6], F32, tag=f"{tag}st{m}",
                               name=f"{tag}st{m}")
                nc.vector.bn_stats(st[:, 0], srcs[m][:, 0:512])
                nc.vector.bn_stats(st[:, 1], srcs[m][:, 512:H])
                nc.vector.bn_aggr(mvb[:, m], st)
            wv = stat.tile([P, TPT], F32, tag=tag + "w", name=tag + "w")
            nc.vector.tensor_scalar(wv, mvb[:, :, 1], 0.5, EPS * 0.5,
                                    op0=ALU.mult, op1=ALU.add)
            rs = emit_rsqrt(wv, tag, n=TPT)
            mrs = stat.tile([P, TPT], F32, tag=tag + "mr", name=tag + "mr")
            nc.vector.tensor_tensor(mrs, mvb[:, :, 0], rs, op=ALU.mult)
            for m in range(TPT):
                nc.vector.tensor_scalar(mrs[:, m:m + 1], mrs[:, m:m + 1],
                                        -1.0, None, op0=ALU.mult)
                nc.scalar.activation(dsts[m], srcs[m],
                                     mybir.ActivationFunctionType.Identity,
                                     bias=mrs[:, m:m + 1], scale=rs[:, m:m + 1])
                if gbc is not None:
                    nc.gpsimd.tensor_mul(dsts[m], dsts[m], gbc)
                    nc.gpsimd.tensor_add(dsts[m], dsts[m], bbc)

        def norm_batched(c, psUs, xrts):
            """Tail epilogue: both m-tiles with shared scalar chains."""
            rs_ = [work.tile([P, H], BF16, tag="r", name=f"r_{m}")
                   for m in range(TPT)]
            ln_level_batched(psUs, rs_, lngbc, lnbbc, "b1")
            for m in range(TPT):
                nc.vector.tensor_add(rs_[m], rs_[m], xrts[m])
            ros = [work.tile([P, H], BF16, tag="ro", name=f"ro_{m}")
                   for m in range(TPT)]
            ln_level_batched(rs_, ros, outgbc, outbbc, "b2")
            for m in range(TPT):
                tok0 = c * TCH + m * P
                nc.sync.dma_start(d_yr[tok0:tok0 + P, :], ros[m])

        NKQ = NKP
        PIPE = min(3, CC)
        n_tiles = CC * TPT - skip_tiles  # live 128-token tiles

        def n_live(c):
            return max(0, min(TPT, n_tiles - c * TPT))

        def chunk_wd(c):
            return P * n_live(c)

        for rep in range(repeat):
            has_q = []
            for c0 in range(min(PIPE, CC)):
                # prologue inputs go via the Act queue (free of weight DMAs
                # now): the Pool queue's SWDGE prep lags at t=0
                xrTt0 = load_xin(c0, eng=nc.scalar)
                has_q.append([g1_gelu(kq, xrTt0, chunk_wd(c0))
                              for kq in range(NKQ)])
            for c in range(CC):
                has_cur = has_q.pop(0)
                xrTt = load_xin(c + PIPE) if c + PIPE < CC else None
                has_next = []
                if xrTt is not None:
                    has_q.append(has_next)
                nl = n_live(c)
                xrts = [
                    xres.tile([P, H], BF16, tag="xrt", name=f"xrt_{m}")
                    for m in range(nl)
                ]
                psUs = [
                    psUp.tile([P, H], F32, tag="psU", name=f"psU_{m}")
                    for m in range(nl)
                ]
                # GEMM2 for m=0 streams through the kp loop (with chunk
                # c+PIPE's GEMM1 interleaved); m=1's GEMM2 runs after, so
                # m=0's epilogue overlaps it. Trailing all-padding tiles
                # (skip_tiles) are skipped entirely.
                act_norm = c >= CC - PIPE
                for kp in range(NKP):
                    if kp == NKP - 2:
                        # residual loads issued late: only consumed by the
                        # identity matmul at the end of the accumulation
                        for m in range(nl):
                            tok0 = c * TCH + m * P
                            nc.sync.dma_start(xrts[m], d_xr[tok0:tok0 + P, :])
                    if xrTt is not None:
                        has_next.append(g1_gelu(kp, xrTt, chunk_wd(c + PIPE)))
                    if nl > 0:
                        g2(kp, has_cur[kp], psUs, ms=(0,))
                if nl > 0:
                    close_m(c, 0, psUs, xrts)
                    norm_m(c, 0, psUs, xrts, act_norm,
                           act_stats=(c == CC - 1))
                if nl > 1:
                    for kp in range(NKP):
                        g2(kp, has_cur[kp], psUs, ms=(1,))
                    close_m(c, 1, psUs, xrts)
                    norm_m(c, 1, psUs, xrts, act_norm)
                for t in range(c, N0, CC):
                    class0_tile(t)

    nc.finalize()
    return nc


_NC_CACHE: dict[tuple, bass.Bass] = {}


def get_nc(cap: int, cap0: int, repeat: int = 1, **flags) -> bass.Bass:
    key = (cap, cap0, repeat, tuple(sorted(flags.items())))
    if key not in _NC_CACHE:
        _NC_CACHE[key] = build_nc(cap, cap0, repeat, **flags)
    return _NC_CACHE[key]


def _round_up(n: int, m: int) -> int:
    return max(m, ((n + m - 1) // m) * m)


def _pack_xrT(x_pad: np.ndarray, g1: str) -> np.ndarray:
    """x_pad: [cap, H] f32. Returns the DRAM layout for d_xrT."""
    cap = x_pad.shape[0]
    if g1 == "fp8":
        CC = cap // TCH
        a = np.ascontiguousarray(x_pad.T) * np.float32(SCL_X)  # [H, cap]
        a = a.reshape(3, 2, P, CC, TCH)            # h = (hp*2 + two)*128 + ki
        a = a.transpose(2, 3, 0, 1, 4)             # [ki, CC, hp, two, TCH]
        return np.ascontiguousarray(a).astype(ml_dtypes.float8_e4m3)
    return np.ascontiguousarray(x_pad.T).astype(ml_dtypes.bfloat16)


def shard_inputs(input_tensor, type_seq, W1, b1, W2, b2, ln_g, ln_b, out_g, out_b):
    """Host-side routing/sharding. Returns (in_maps, core_tokens, zero_splits,
    cap, cap0, flags)."""
    B, L, _H = input_tensor.shape
    assert _H == H, f"kernel hardcodes d_model={H}, got {_H}"
    x = np.ascontiguousarray(np.asarray(input_tensor, dtype=np.float32)).reshape(B * L, H)
    ts_flat = np.asarray(type_seq).reshape(-1).astype(np.int64)
    NB = W1.shape[0]
    per_expert = max(1, NCORES // NB)

    b1 = np.asarray(b1, dtype=np.float32)
    b2 = np.asarray(b2, dtype=np.float32)
    ln_g = np.asarray(ln_g, dtype=np.float32)
    ln_b = np.asarray(ln_b, dtype=np.float32)
    out_g = np.asarray(out_g, dtype=np.float32)
    out_b = np.asarray(out_b, dtype=np.float32)
    flags = {
        "g1": G1_DT,
        "g2": G2_DT,
        "triv_b1": bool((b1 == 0).all()),
        "triv_b2": bool((b2 == 0).all()),
        "triv_aff1": bool((ln_g == 1).all() and (ln_b == 0).all()),
        "triv_aff2": bool((out_g == 1).all() and (out_b == 0).all()),
    }

    core_tokens = []
    core_expert = []
    for e in range(NB):
        toks = np.nonzero(ts_flat == e + 1)[0]
        for s in np.array_split(toks, per_expert):
            core_tokens.append(s)
            core_expert.append(e)
    while len(core_tokens) < NCORES:  # NB not dividing NCORES: idle cores
        core_tokens.append(np.zeros(0, dtype=np.int64))
        core_expert.append(0)
    zero_splits = np.array_split(np.nonzero(ts_flat == 0)[0], NCORES)

    max_real = max(len(t) for t in core_tokens)
    cap = _round_up(max_real, TCH)
    cap0 = _round_up(max(len(z) for z in zero_splits), P)
    # trailing 128-token tiles that are pure padding on every core
    flags["skip_tiles"] = (cap - _round_up(max_real, P)) // P

    g1dt = ml_dtypes.float8_e4m3 if flags["g1"] == "fp8" else ml_dtypes.bfloat16
    g2dt = ml_dtypes.float8_e4m3 if flags["g2"] == "fp8" else ml_dtypes.bfloat16
    s_w1 = SCL_W1 if flags["g1"] == "fp8" else 1.0
    s_w2 = SCL_W2 if flags["g2"] == "fp8" else 1.0
    ident = (np.eye(P, dtype=np.float32) * np.float32(s_w2)).astype(ml_dtypes.bfloat16)

    in_maps = []
    for cidx in range(NCORES):
        toks = core_tokens[cidx]
        e = core_expert[cidx]
        z = zero_splits[cidx]
        xr = np.zeros((cap, H), np.float32)
        xr[: len(toks)] = x[toks]
        x0 = np.zeros((cap0, H), ml_dtypes.bfloat16)
        x0[: len(z)] = x[z]
        m = {
            "xr": xr.astype(ml_dtypes.bfloat16),
            "xrT": _pack_xrT(xr, flags["g1"]),
            "x0": x0,
            "ident": ident,
            "w1": (np.asarray(W1[e], dtype=np.float32) * np.float32(s_w1)).astype(g1dt),
            "w2": (np.asarray(W2[e], dtype=np.float32) * np.float32(s_w2)).astype(g2dt),
        }
        if not flags["triv_b1"]:
            m["b1"] = np.ascontiguousarray(b1[e])
        if not flags["triv_b2"]:
            m["b2"] = (b2[e] * np.float32(s_w2)).astype(g2dt)
        if not flags["triv_aff1"]:
            m["lng"] = np.ascontiguousarray(ln_g[e])
            m["lnb"] = np.ascontiguousarray(ln_b[e])
        if not flags["triv_aff2"]:
            m["outg"] = np.ascontiguousarray(out_g)
            m["outb"] = np.ascontiguousarray(out_b)
        in_maps.append(m)
    return in_maps, core_tokens, zero_splits, cap, cap0, flags


def unshard_output(results, core_tokens, zero_splits, shape, dtype):
    B, L, _H = shape
    out = np.empty((B * L, H), np.float32)
    for c in range(NCORES):
        toks = core_tokens[c]
        z = zero_splits[c]
        if len(toks):
            out[toks] = results[c]["yr"][: len(toks)].astype(np.float32)
        if len(z):
            out[z] = results[c]["y0"][: len(z)].astype(np.float32)
    return out.reshape(B, L, H).astype(dtype, copy=False)


def kernel(input_tensor, type_seq, W1, b1, W2, b2, ln_g, ln_b, out_g, out_b):
    in_maps, core_tokens, zero_splits, cap, cap0, flags = shard_inputs(
        input_tensor, type_seq, W1, b1, W2, b2, ln_g, ln_b, out_g, out_b
    )
    nc = get_nc(cap, cap0, **flags)
    res = run_bass_kernel_spmd(nc, in_maps, core_ids=list(range(NCORES)))
    return unshard_output(
        res.results, core_tokens, zero_splits, input_tensor.shape,
        np.asarray(input_tensor).dtype,
    )
